# revision 35
# baseline (speedup 1.0000x reference)
import os
import numpy as np

import concourse.bass as bass
import concourse.bacc as bacc
import concourse.mybir as mybir
import concourse.tile as tile
from concourse import bass_utils
from concourse.masks import make_identity

F32 = mybir.dt.float32
F16 = mybir.dt.float16
AF = mybir.ActivationFunctionType

B, D, N, H = 4, 256, 2048, 4
HD = D // H
P = 128
NQ = N // 2
NCORES = 8
NMT = N // P

OFF_WQ, OFF_WK, OFF_WV = 0, 512, 1024
OFF_W1A, OFF_WM, OFF_W1B, OFF_W2 = 1536, 2560, 3072, 4096
WCOLS = 5120
XCOLS = 2 * NQ
SCOLS = 2 * N

LAST_RESULTS = None


def build_program(reps: int = 1):
    nc = bacc.Bacc(None, target_bir_lowering=False)

    wpk = nc.dram_tensor("wpk", [P, WCOLS], F16, kind="ExternalInput")
    xpk = nc.dram_tensor("xpk", [P, XCOLS], F16, kind="ExternalInput")
    spk = nc.dram_tensor("spk", [P, SCOLS], F16, kind="ExternalInput")
    edgeT = nc.dram_tensor("edgeT", [N, NQ], F16, kind="ExternalInput")
    bpk = nc.dram_tensor("bpk", [P, 14], F32, kind="ExternalInput")
    out = nc.dram_tensor("out", [D, NQ], F32, kind="ExternalOutput")

    with tile.TileContext(nc) as tc:
        _loop = tc.For_i(0, reps, 1) if reps > 1 else None
        if _loop is not None:
            _loop.__enter__()
        with (
            tc.tile_pool(name="const", bufs=1) as cp,
            tc.tile_pool(name="w", bufs=1) as wp,
            tc.tile_pool(name="acts", bufs=1) as ap,
        ):
            ident16 = cp.tile([P, P], F16)
            make_identity(nc, ident16)
            ones16 = cp.tile([P, 1], F16)
            nc.vector.memset(ones16, 1.0)
            ones_row = cp.tile([1, HD], F16)
            nc.vector.memset(ones_row, 1.0)
            bias = cp.tile([P, 14], F32)
            nc.sync.dma_start(out=bias[:, :], in_=bpk[:, :])

            wx_sb = wp.tile([P, WCOLS + XCOLS + SCOLS], F16)
            nc.sync.dma_start(out=wx_sb[:, 0:XCOLS], in_=xpk[:, :])
            nc.sync.dma_start(out=wx_sb[:, XCOLS:XCOLS + WCOLS], in_=wpk[:, :])
            nc.sync.dma_start(out=wx_sb[:, XCOLS + WCOLS:], in_=spk[:, :])

            def wview(off, ncols, nk):
                return wx_sb[:, off:off + nk * ncols].rearrange(
                    "p (k c) -> p k c", k=nk)

            x_sb = wview(0, NQ, 2)
            wq_sb = wview(XCOLS + OFF_WQ, D, 2)
            wk_sb = wview(XCOLS + OFF_WK, D, 2)
            wv_sb = wview(XCOLS + OFF_WV, D, 2)
            w1a_sb = wview(XCOLS + OFF_W1A, 2 * D, 2)
            wm_sb = wview(XCOLS + OFF_WM, D, 2)
            w1b_sb = wview(XCOLS + OFF_W1B, 2 * D, 2)
            w2_sb = wview(XCOLS + OFF_W2, D, 4)
            src_sb = wview(XCOLS + WCOLS, N, 2)

            q_sb = ap.tile([P, 2, NQ], F16)
            k_sb = ap.tile([P, 2, N], F16)
            vt_sb = ap.tile([P, 2, 4, 2, 4, HD], F16)
            msg_sb = ap.tile([P, 2, NQ], F16)
            msg2_sb = ap.tile([P, 2, NQ], F16)
            h1_sb = ap.tile([P, 4, NQ], F16)

            with (
                tc.tile_pool(name="pp", bufs=2, space="PSUM") as pp,
                tc.tile_pool(name="ptv", bufs=2, space="PSUM") as ptv,
                tc.tile_pool(name="vchunk", bufs=2) as vcp,
            ):
                for dt_ in range(2):
                    for nchk in range(2):
                        ps = pp.tile([P, 512], F32, tag="ps")
                        for kk in range(2):
                            nc.tensor.matmul(
                                ps[:, :],
                                wq_sb[:, kk, dt_ * P:(dt_ + 1) * P],
                                x_sb[:, kk, nchk * 512:(nchk + 1) * 512],
                                start=(kk == 0), stop=(kk == 1))
                        nc.scalar.activation(
                            q_sb[:, dt_, nchk * 512:(nchk + 1) * 512], ps[:, :],
                            AF.Identity, bias=bias[:, dt_:dt_ + 1])
                for dt_ in range(2):
                    for nchk in range(4):
                        ps = pp.tile([P, 512], F32, tag="ps")
                        for kk in range(2):
                            nc.tensor.matmul(
                                ps[:, :],
                                wk_sb[:, kk, dt_ * P:(dt_ + 1) * P],
                                src_sb[:, kk, nchk * 512:(nchk + 1) * 512],
                                start=(kk == 0), stop=(kk == 1))
                        nc.scalar.activation(
                            k_sb[:, dt_, nchk * 512:(nchk + 1) * 512], ps[:, :],
                            AF.Identity, bias=bias[:, 2 + dt_:3 + dt_])
                for dt_ in range(2):
                    for nchk in range(4):
                        ps = pp.tile([P, 512], F32, tag="ps")
                        for kk in range(2):
                            nc.tensor.matmul(
                                ps[:, :],
                                wv_sb[:, kk, dt_ * P:(dt_ + 1) * P],
                                src_sb[:, kk, nchk * 512:(nchk + 1) * 512],
                                start=(kk == 0), stop=(kk == 1))
                        vc = vcp.tile([P, 512], F16)
                        nc.scalar.activation(
                            vc[:, :], ps[:, :],
                            AF.Identity, bias=bias[:, 4 + dt_:5 + dt_])
                        for hh in range(2):
                            for j in range(4):
                                pt = ptv.tile([P, HD], F16, tag="pt")
                                nc.tensor.transpose(
                                    pt[:, :],
                                    vc[hh * HD:(hh + 1) * HD, j * P:(j + 1) * P],
                                    ident16[hh * HD:(hh + 1) * HD,
                                            hh * HD:(hh + 1) * HD])
                                nc.vector.tensor_copy(
                                    vt_sb[:, dt_, nchk, hh, j, :], pt[:, :])

            with (
                tc.tile_pool(name="pscore", bufs=2, space="PSUM") as pscore,
                tc.tile_pool(name="pmsg", bufs=1, space="PSUM") as pmsg,
                tc.tile_pool(name="pden", bufs=1, space="PSUM") as pden,
                tc.tile_pool(name="edgep", bufs=1) as edgep,
                tc.tile_pool(name="up", bufs=3) as up,
                tc.tile_pool(name="accp", bufs=2) as accp,
                tc.tile_pool(name="rdp", bufs=2) as rdp,
                tc.tile_pool(name="outp", bufs=2) as outp,
            ):
                GROUPS = ((0, 3), (3, 3), (6, 3), (9, 3), (12, 3), (15, 1))
                edge_tiles = []
                for c in range(2):
                    edge_t = edgep.tile([P, NMT, 512], F16, tag=f"edge{c}")
                    for g in range(4):
                        nc.sync.dma_start(
                            out=edge_t[:, 4 * g:4 * g + 4, :],
                            in_=edgeT[4 * g * P:4 * (g + 1) * P,
                                      c * 512:(c + 1) * 512].rearrange(
                                          "(t p) n -> p t n", p=P))
                    edge_tiles.append(edge_t)
                for c in range(2):
                    edge_t = edge_tiles[c]
                    pend = None
                    for h in range(H + 1):
                        cur = None
                        if h < H:
                            hb, ht = HD * (h % 2), h // 2
                            qh = q_sb[hb:hb + HD, ht, c * 512:(c + 1) * 512]
                            u = up.tile([P, NMT, 512], F16)
                            acc = accp.tile([P, 3, 512], F16)
                            for gi, (g0, gn) in enumerate(GROUPS):
                                ps = pscore.tile([P, 3, 512], F32, tag="ps2")
                                for j in range(gn):
                                    mt = g0 + j
                                    nc.tensor.matmul(
                                        ps[:, j, :],
                                        k_sb[hb:hb + HD, ht,
                                             mt * P:(mt + 1) * P],
                                        qh, start=True, stop=True)
                                nc.scalar.activation(
                                    u[:, g0:g0 + gn, :], ps[:, 0:gn, :],
                                    AF.Exp, scale=0.125)
                                if gi == 1:
                                    nc.vector.tensor_add(acc[:, :, :],
                                                         u[:, 0:3, :],
                                                         u[:, 3:6, :])
                                    nc.vector.tensor_mul(u[:, 0:3, :],
                                                         u[:, 0:3, :],
                                                         edge_t[:, 0:3, :])
                                    nc.vector.tensor_mul(u[:, 3:6, :],
                                                         u[:, 3:6, :],
                                                         edge_t[:, 3:6, :])
                                elif gi in (2, 3):
                                    nc.vector.tensor_add(acc[:, :, :],
                                                         acc[:, :, :],
                                                         u[:, g0:g0 + 3, :])
                                    nc.vector.tensor_mul(
                                        u[:, g0:g0 + 3, :], u[:, g0:g0 + 3, :],
                                        edge_t[:, g0:g0 + 3, :])
                                elif gi == 4:
                                    nc.vector.tensor_add(acc[:, :, :],
                                                         acc[:, :, :],
                                                         u[:, g0:g0 + 3, :])
                                    nc.gpsimd.tensor_mul(
                                        u[:, g0:g0 + 3, :], u[:, g0:g0 + 3, :],
                                        edge_t[:, g0:g0 + 3, :])
                                elif gi == 5:
                                    nc.vector.tensor_add(acc[:, 0, :],
                                                         acc[:, 0, :],
                                                         u[:, 15, :])
                                    nc.vector.tensor_add(acc[:, 0, :],
                                                         acc[:, 0, :],
                                                         acc[:, 1, :])
                                    nc.vector.tensor_add(acc[:, 0, :],
                                                         acc[:, 0, :],
                                                         acc[:, 2, :])
                                    nc.gpsimd.tensor_mul(
                                        u[:, 15:16, :], u[:, 15:16, :],
                                        edge_t[:, 15:16, :])
                            cur = (u, acc, hb, ht)
                        if pend is not None:
                            u_p, acc_p, hb_p, ht_p = pend
                            dnb = pden.tile([P, 512], F32, tag="dnb")
                            nc.tensor.matmul(dnb[0:1, :], ones16[:, :],
                                             acc_p[:, 0, :],
                                             start=True, stop=True)
                            rden = rdp.tile([1, 512], F16, tag="rden")
                            with nc.allow_low_precision("fp16 recip of den"):
                                nc.vector.reciprocal(rden[:, :], dnb[0:1, :])
                            mps = pmsg.tile([HD, 512], F32, tag="msg")
                            for mt in range(NMT):
                                nc.tensor.matmul(
                                    mps[:, :],
                                    vt_sb[:, (2 * ht_p + hb_p // HD) // 2,
                                          mt // 4, hb_p // HD, mt % 4, :],
                                    u_p[:, mt, :],
                                    start=(mt == 0), stop=(mt == NMT - 1))
                            nc.tensor.matmul(dnb[HD:2 * HD, :], ones_row[:, :],
                                             rden[:, :], start=True, stop=True,
                                             skip_group_check=True)
                            rdbc = rdp.tile([HD, 512], F32, tag="rdbc")
                            nc.vector.tensor_copy(rdbc[:, :], dnb[HD:2 * HD, :])
                            nc.vector.tensor_mul(
                                msg_sb[hb_p:hb_p + HD, ht_p,
                                       c * 512:(c + 1) * 512],
                                mps[:, :], rdbc[:, :])
                        pend = cur

                    for sub in range(2):
                        s0 = c * 512 + sub * 256
                        r = slice(s0, s0 + 256)
                        for dt_ in range(2):
                            ps = pscore.tile([P, 256], F32, tag="ps2")
                            for kk in range(2):
                                nc.tensor.matmul(
                                    ps[:, :],
                                    wm_sb[:, kk, dt_ * P:(dt_ + 1) * P],
                                    msg_sb[:, kk, r],
                                    start=(kk == 0), stop=(kk == 1))
                            nc.vector.tensor_scalar_add(
                                msg2_sb[:, dt_, r], ps[:, :],
                                bias[:, 6 + dt_:7 + dt_])
                        for dt_ in range(4):
                            ps = pscore.tile([P, 256], F32, tag="ps2")
                            for kk in range(2):
                                nc.tensor.matmul(
                                    ps[:, :],
                                    w1a_sb[:, kk, dt_ * P:(dt_ + 1) * P],
                                    x_sb[:, kk, r],
                                    start=(kk == 0), stop=False)
                            for kk in range(2):
                                nc.tensor.matmul(
                                    ps[:, :],
                                    w1b_sb[:, kk, dt_ * P:(dt_ + 1) * P],
                                    msg2_sb[:, kk, r],
                                    start=False, stop=(kk == 1))
                            nc.vector.tensor_scalar(
                                h1_sb[:, dt_, r], ps[:, :],
                                bias[:, 8 + dt_:9 + dt_], 0.0,
                                op0=mybir.AluOpType.add,
                                op1=mybir.AluOpType.max)
                        for dt_ in range(2):
                            ps = pscore.tile([P, 256], F32, tag="ps2")
                            for kk in range(4):
                                nc.tensor.matmul(
                                    ps[:, :],
                                    w2_sb[:, kk, dt_ * P:(dt_ + 1) * P],
                                    h1_sb[:, kk, r],
                                    start=(kk == 0), stop=(kk == 3))
                            oc = outp.tile([P, 256], F32)
                            nc.vector.tensor_scalar_add(
                                oc[:, :], ps[:, :],
                                bias[:, 12 + dt_:13 + dt_])
                            nc.sync.dma_start(
                                out=out[dt_ * P:(dt_ + 1) * P, r],
                                in_=oc[:, :])
        if _loop is not None:
            _loop.__exit__(None, None, None)
    nc.finalize()
    return nc


def _pack_rows(a, nk):
    c = a.shape[1]
    return np.ascontiguousarray(
        a.reshape(nk, P, c).transpose(1, 0, 2).reshape(P, nk * c))


def prepare_in_maps(inputs):
    x = np.asarray(inputs["x"], np.float32)
    source = np.asarray(inputs["source"], np.float32)
    edge = np.asarray(inputs["edge"], np.float32)
    Wq, bq = np.asarray(inputs["Wq"], np.float32), np.asarray(inputs["bq"], np.float32)
    Wk, bk = np.asarray(inputs["Wk"], np.float32), np.asarray(inputs["bk"], np.float32)
    Wv, bv = np.asarray(inputs["Wv"], np.float32), np.asarray(inputs["bv"], np.float32)
    Wm, bm = np.asarray(inputs["Wm"], np.float32), np.asarray(inputs["bm"], np.float32)
    W1, b1 = np.asarray(inputs["W1"], np.float32), np.asarray(inputs["b1"], np.float32)
    W2, b2 = np.asarray(inputs["W2"], np.float32), np.asarray(inputs["b2"], np.float32)

    perm = np.array([(j % HD) * H + j // HD for j in range(D)])

    f16 = np.float16
    wpk = np.concatenate([
        _pack_rows(Wq[perm].T.astype(f16), 2),
        _pack_rows(Wk[perm].T.astype(f16), 2),
        _pack_rows(Wv[perm].T.astype(f16), 2),
        _pack_rows(W1[:, :D].T.astype(f16), 2),
        _pack_rows(Wm[:, perm].T.astype(f16), 2),
        _pack_rows(W1[:, D:].T.astype(f16), 2),
        _pack_rows(W2.T.astype(f16), 4),
    ], axis=1)
    bpk = np.stack([
        bq[perm][:P], bq[perm][P:], bk[perm][:P], bk[perm][P:],
        bv[perm][:P], bv[perm][P:], bm[:P], bm[P:],
        b1[:P], b1[P:2 * P], b1[2 * P:3 * P], b1[3 * P:],
        b2[:P], b2[P:],
    ], axis=1).astype(np.float32)
    bpk = np.ascontiguousarray(bpk)

    shared = {"wpk": wpk, "bpk": bpk}
    in_maps = []
    for c in range(NCORES):
        b, half = c // 2, c % 2
        sl = slice(half * NQ, (half + 1) * NQ)
        in_maps.append({
            "xpk": _pack_rows(x[b, :, sl].astype(f16), 2),
            "spk": _pack_rows(source[b].astype(f16), 2),
            "edgeT": np.ascontiguousarray(edge[b, sl, :].T.astype(f16)),
            **shared,
        })
    return in_maps


def kernel(**inputs) -> np.ndarray:
    global LAST_RESULTS
    in_maps = prepare_in_maps(inputs)
    nc = build_program()
    LAST_RESULTS = bass_utils.run_bass_kernel_spmd(
        nc, in_maps, core_ids=list(range(NCORES)),
        trace=os.environ.get("BASS_KERNEL_TRACE", "0") == "1",
    )

    y = np.empty((B, D, N), np.float32)
    for c in range(NCORES):
        b, half = c // 2, c % 2
        y[b, :, half * NQ:(half + 1) * NQ] = LAST_RESULTS.results[c]["out"]
    return y
